# revision 20
# baseline (speedup 1.0000x reference)
"""Causal self-attention on 8 trn2 NeuronCores.

Sharding: tensor-parallel over heads (2 heads per core, both batches).
Each core computes Q/K/V projections for its heads (column-parallel),
causal attention, and a row-parallel partial of the output projection.
Host sums the 8 partials and adds the bias terms (bo, and bv@Wo which
is exact because softmax rows sum to 1).

All matmuls run in fp32r (full PE rate, ~1.5e-4 rms rounding);
softmax (exp / sum / reciprocal) is fp32.

Layout choices (partition dim first):
  xt   : x transposed -> (model 128-blocks on partitions, seq free)
  Qt/Kt: (head_dim on partitions, seq free)        [proj lhsT = W block]
  V    : (seq keys on partitions, head_dim free)   [proj lhsT = xt block]
  scores: St = (keys, queries) tiles = Kt_blk.T @ Qt_chunk
  exp(St) feeds P@V directly:  attnT = V_blk.T @ exp  (head_dim, queries)
  denominator: ones(128,1).T @ exp accumulated over key tiles
  out-proj: lhsT = attnT block, rhs = Wo rows slice -> (queries, model)
"""

import os

import numpy as np

import concourse.bass as bass
import concourse.mybir as mybir
import concourse.tile as tile
from concourse import bacc
from concourse.bass_utils import run_bass_kernel_spmd

F32 = mybir.dt.float32
F32R = mybir.dt.float32r
AF = mybir.ActivationFunctionType
ALU = mybir.AluOpType

B = 2
S = 2048
D = 2048
H = 16
DH = 128
NCORES = 8
HPC = H // NCORES  # heads per core = 2
KT = D // 128  # 16 contraction tiles for projections
NQC = S // 512  # 4 query chunks per sequence
NST = S // 128  # 16 key tiles per sequence
SCALE = 1.0 / np.sqrt(DH)
NEG = -1e9

_NC_CACHE = {}


def _build():
    nc = bacc.Bacc(None, target_bir_lowering=False, debug=False)

    xt = nc.dram_tensor("xt", [B, KT, 128, S], F32R, kind="ExternalInput")
    wq = nc.dram_tensor("wq", [KT, 128, HPC * DH], F32R, kind="ExternalInput")
    wk = nc.dram_tensor("wk", [KT, 128, HPC * DH], F32R, kind="ExternalInput")
    wv = nc.dram_tensor("wv", [KT, 128, HPC * DH], F32R, kind="ExternalInput")
    wo = nc.dram_tensor("wo", [HPC, 128, D], F32R, kind="ExternalInput")
    bq2 = nc.dram_tensor("bq2", [HPC, 128], F32, kind="ExternalInput")
    bk2 = nc.dram_tensor("bk2", [HPC, 128], F32, kind="ExternalInput")
    mblk = nc.dram_tensor("mblk", [128, 128], F32, kind="ExternalInput")
    onem = nc.dram_tensor("onem", [128, 128], F32R, kind="ExternalInput")
    out = nc.dram_tensor("out", [B, S, D], F32, kind="ExternalOutput")

    with tile.TileContext(nc) as tc:
        with (
            tc.tile_pool(name="const", bufs=1) as constp,
            tc.tile_pool(name="xtp", bufs=20) as xtp,
            tc.tile_pool(name="qkv", bufs=1) as qkvp,
            tc.tile_pool(name="expp", bufs=5) as expp,
            tc.tile_pool(name="attnp", bufs=5) as attnp,
            tc.tile_pool(name="ostp", bufs=4) as ostp,
            # PSUM bank budget (8): st 3 + attn 2 + op 3
            tc.tile_pool(name="ps_st", bufs=3, space="PSUM") as ps_st,
            tc.tile_pool(name="ps_at", bufs=2, space="PSUM") as ps_at,
            tc.tile_pool(name="ps_op", bufs=3, space="PSUM") as ps_op,
        ):
            wq_t = constp.tile([128, KT, HPC * DH], F32R, tag="wq")
            wk_t = constp.tile([128, KT, HPC * DH], F32R, tag="wk")
            wv_t = constp.tile([128, KT, HPC * DH], F32R, tag="wv")
            wo_t = constp.tile([128, HPC, D], F32R, tag="wo")
            for k in range(KT):
                nc.gpsimd.dma_start(wq_t[:, k, :], wq[k])
                nc.gpsimd.dma_start(wk_t[:, k, :], wk[k])
                nc.gpsimd.dma_start(wv_t[:, k, :], wv[k])
            nc.gpsimd.dma_start(wo_t[:], wo.rearrange("h p d -> p h d"))
            bq_t = constp.tile([128, HPC], F32, tag="bq")
            bk_t = constp.tile([128, HPC], F32, tag="bk")
            nc.gpsimd.dma_start(bq_t[:], bq2.rearrange("h p -> p h"))
            nc.gpsimd.dma_start(bk_t[:], bk2.rearrange("h p -> p h"))
            mask_t = constp.tile([128, 128], F32, tag="mask")
            nc.gpsimd.dma_start(mask_t[:], mblk[:])
            ones_m = constp.tile([128, 128], F32R, tag="ones_m")
            nc.gpsimd.dma_start(ones_m[:], onem[:])
            warm_t = constp.tile([128, 1], F32, tag="warm")
            nc.scalar.activation(warm_t[:], mask_t[:, 0:1], AF.Exp, scale=0.0)

            units = []
            for b in range(B):
                # ---------------- projections for batch b ----------------
                qt = qkvp.tile([128, HPC, S], F32R, tag="qt", name=f"qt{b}")
                ktt = qkvp.tile([128, HPC, S], F32R, tag="ktt",
                                name=f"ktt{b}")
                vt = qkvp.tile([128, NST, HPC * DH], F32R, tag="vt",
                               name=f"vt{b}")
                def outproj(qc, at_sb, b=b):
                    for qs in range(4):
                        row0 = qc * 512 + qs * 128
                        for nch in range(4):
                            ps = ps_op.tile([128, 512], F32, tag="ps",
                                            name="op_ps")
                            for h in range(HPC):
                                nc.tensor.matmul(
                                    ps[:],
                                    at_sb[h][:, qs * 128 : (qs + 1) * 128],
                                    wo_t[:, h, nch * 512 : (nch + 1) * 512],
                                    start=(h == 0),
                                    stop=(h == HPC - 1),
                                )
                            o_t = ostp.tile([128, 512], F32, tag="ost",
                                            name="o_t")
                            if nch % 2 == 0:
                                nc.vector.tensor_copy(o_t[:], ps[:])
                            else:
                                nc.scalar.copy(o_t[:], ps[:])
                            nc.gpsimd.dma_start(
                                out[
                                    b,
                                    row0 : row0 + 128,
                                    nch * 512 : (nch + 1) * 512,
                                ],
                                o_t[:],
                            )


                def p_chunk(c, b=b, qt=qt, ktt=ktt, vt=vt):
                    xts = []
                    for k in range(KT):
                        x_t = xtp.tile([128, 512], F32R, tag="xt",
                                       name="x_t")
                        nc.sync.dma_start(
                            x_t[:], xt[b, k, :, c * 512 : (c + 1) * 512]
                        )
                        xts.append(x_t)
                    for h in range(HPC):
                        for w_t, dst, bias_t, use_act in (
                            (wq_t, qt, bq_t, True),
                            (wk_t, ktt, bk_t, False),
                        ):
                            pool = ps_at if use_act else ps_op
                            ps = pool.tile([128, 512], F32, tag="ps",
                                           name="qk_ps")
                            for k in range(KT):
                                nc.tensor.matmul(
                                    ps[:],
                                    w_t[:, k, h * DH : (h + 1) * DH],
                                    xts[k][:],
                                    start=(k == 0),
                                    stop=(k == KT - 1),
                                )
                            dsl = dst[:, h, c * 512 : (c + 1) * 512]
                            if use_act:
                                nc.scalar.add(dsl, ps[:], bias_t[:, h : h + 1])
                            else:
                                nc.vector.tensor_scalar_add(
                                    dsl, ps[:], bias_t[:, h : h + 1]
                                )
                    for s in range(4):
                        ps = ps_st.tile([128, HPC * DH], F32, tag="st",
                                        name="v_ps")
                        for k in range(KT):
                            nc.tensor.matmul(
                                ps[:],
                                xts[k][:, s * 128 : (s + 1) * 128],
                                wv_t[:, k, :],
                                start=(k == 0),
                                stop=(k == KT - 1),
                            )
                        if s % 2 == 0:
                            nc.vector.tensor_copy(vt[:, c * 4 + s, :], ps[:])
                        else:
                            nc.scalar.copy(vt[:, c * 4 + s, :], ps[:])

                def a_chunk(qc, qt=qt, ktt=ktt, vt=vt):
                    n_kt = 4 * (qc + 1)
                    qsl = slice(qc * 512, (qc + 1) * 512)
                    at_sb = []
                    for h in range(HPC):
                        hsl = slice(h * DH, (h + 1) * DH)
                        attn_ps = ps_at.tile([128, 512], F32, tag="ps",
                                             name="attn_ps")
                        rbden_ps = ps_op.tile([128, 512], F32, tag="ps",
                                              name="rbden_ps")
                        for i in range(n_kt):
                            st = ps_st.tile([128, 512], F32, tag="st",
                                            name="st")
                            expt = expp.tile([128, 512], F32R, tag="exp",
                                             name="expt")
                            lo0 = 128 * (i - 4 * qc) if i >= 4 * qc else 0
                            nc.tensor.matmul(
                                st[:, lo0:],
                                ktt[:, h, i * 128 : (i + 1) * 128],
                                qt[:, h, qc * 512 + lo0 : (qc + 1) * 512],
                                start=True,
                                stop=True,
                            )
                            if i >= 4 * qc:  # diagonal tile
                                lo = 128 * (i - 4 * qc)
                                nc.vector.tensor_tensor(
                                    st[:, lo : lo + 128],
                                    st[:, lo : lo + 128],
                                    mask_t[:],
                                    op=ALU.add,
                                )
                            else:
                                lo = 0
                            nc.scalar.activation(
                                expt[:, lo:], st[:, lo:], AF.Exp, scale=SCALE
                            )
                            nc.tensor.matmul(
                                attn_ps[:, lo:],
                                vt[:, i, hsl],
                                expt[:, lo:],
                                start=(i == 0),
                                stop=(i == n_kt - 1),
                            )
                            nc.tensor.matmul(
                                rbden_ps[:, lo:],
                                ones_m[:],
                                expt[:, lo:],
                                start=(i == 0),
                                stop=(i == n_kt - 1),
                            )
                        rc_sb = attnp.tile([128, 512], F32, tag="rc",
                                           name="rc_sb")
                        nc.vector.reciprocal_approx_fast(
                            out=rc_sb[:], in_=rbden_ps[:]
                        )
                        a_sb = attnp.tile([128, 512], F32R, tag="attnT",
                                          name="a_sb")
                        nc.vector.tensor_tensor(
                            a_sb[:], attn_ps[:], rc_sb[:], op=ALU.mult
                        )
                        at_sb.append(a_sb)
                    return at_sb

                units.append((p_chunk, a_chunk, outproj))

            pending = None
            for p_fn, a_fn, op_fn in units:
                for c in range(NQC):
                    p_fn(c)
                    at = a_fn(c)
                    if pending is not None:
                        pending[0](pending[1], pending[2])
                    pending = (op_fn, c, at)
            pending[0](pending[1], pending[2])
    nc.compile()
    return nc


def _get_nc():
    if "nc" not in _NC_CACHE:
        _NC_CACHE["nc"] = _build()
    return _NC_CACHE["nc"]


def kernel(x, mask, Wq, bq, Wk, bk, Wv, bv, Wo, bo):
    x = np.asarray(x, dtype=np.float32)
    Wq = np.asarray(Wq, dtype=np.float32)
    Wk = np.asarray(Wk, dtype=np.float32)
    Wv = np.asarray(Wv, dtype=np.float32)
    Wo = np.asarray(Wo, dtype=np.float32)
    bq = np.asarray(bq, dtype=np.float32)
    bk = np.asarray(bk, dtype=np.float32)
    bv = np.asarray(bv, dtype=np.float32)
    bo = np.asarray(bo, dtype=np.float32)

    nc = _get_nc()

    xt_np = np.ascontiguousarray(
        x.transpose(0, 2, 1).reshape(B, KT, 128, S)
    )
    kl = np.arange(128)
    mblk = np.where(kl[:, None] <= kl[None, :], 0.0, NEG).astype(np.float32)

    in_maps = []
    for c in range(NCORES):
        cols = slice(c * HPC * DH, (c + 1) * HPC * DH)
        in_maps.append(
            {
                "xt": xt_np,
                "wq": np.ascontiguousarray(Wq[:, cols]).reshape(
                    KT, 128, HPC * DH
                ),
                "wk": np.ascontiguousarray(Wk[:, cols]).reshape(
                    KT, 128, HPC * DH
                ),
                "wv": np.ascontiguousarray(Wv[:, cols]).reshape(
                    KT, 128, HPC * DH
                ),
                "wo": np.ascontiguousarray(Wo[cols, :]).reshape(HPC, 128, D),
                "bq2": np.ascontiguousarray(bq[cols]).reshape(HPC, 128),
                "bk2": np.ascontiguousarray(bk[cols]).reshape(HPC, 128),
                "mblk": mblk,
                "onem": np.ones((128, 128), dtype=np.float32),
            }
        )

    trace = bool(int(os.environ.get("BASS_ATTN_TRACE", "0")))
    try:
        res = run_bass_kernel_spmd(
            nc, in_maps, core_ids=list(range(NCORES)), trace=trace
        )
    except Exception:
        # transient device errors (e.g. a wedged core from a prior run)
        # usually clear on retry
        res = run_bass_kernel_spmd(
            nc, in_maps, core_ids=list(range(NCORES)), trace=trace
        )
    if trace:
        _NC_CACHE["last_result"] = res

    acc = res.results[0]["out"].astype(np.float32)
    for c in range(1, NCORES):
        acc += res.results[c]["out"]
    # bv's effect: softmax rows sum to 1, so attn = attn_nobv + bv per head
    # -> out += bv @ Wo (exact). bo added directly.
    corr = (bv.astype(np.float64) @ Wo.astype(np.float64)) + bo.astype(
        np.float64
    )
    acc += corr.astype(np.float32)
    return acc


# revision 21
# speedup vs baseline: 1.0356x; 1.0356x over previous
"""Causal self-attention on 8 trn2 NeuronCores.

Sharding: tensor-parallel over heads (2 heads per core, both batches).
Each core computes Q/K/V projections for its heads (column-parallel),
causal attention, and a row-parallel partial of the output projection.
Host sums the 8 partials and adds the bias terms (bo, and bv@Wo which
is exact because softmax rows sum to 1).

All matmuls run in fp32r (full PE rate, ~1.5e-4 rms rounding);
softmax (exp / sum / reciprocal) is fp32.

Layout choices (partition dim first):
  xt   : x transposed -> (model 128-blocks on partitions, seq free)
  Qt/Kt: (head_dim on partitions, seq free)        [proj lhsT = W block]
  V    : (seq keys on partitions, head_dim free)   [proj lhsT = xt block]
  scores: St = (keys, queries) tiles = Kt_blk.T @ Qt_chunk
  exp(St) feeds P@V directly:  attnT = V_blk.T @ exp  (head_dim, queries)
  denominator: ones(128,1).T @ exp accumulated over key tiles
  out-proj: lhsT = attnT block, rhs = Wo rows slice -> (queries, model)
"""

import os

import numpy as np

import concourse.bass as bass
import concourse.mybir as mybir
import concourse.tile as tile
from concourse import bacc
from concourse.bass_utils import run_bass_kernel_spmd

F32 = mybir.dt.float32
F32R = mybir.dt.float32r
AF = mybir.ActivationFunctionType
ALU = mybir.AluOpType

B = 2
S = 2048
D = 2048
H = 16
DH = 128
NCORES = 8
HPC = H // NCORES  # heads per core = 2
KT = D // 128  # 16 contraction tiles for projections
NQC = S // 512  # 4 query chunks per sequence
NST = S // 128  # 16 key tiles per sequence
SCALE = 1.0 / np.sqrt(DH)
NEG = -1e9

_NC_CACHE = {}


def _build():
    nc = bacc.Bacc(None, target_bir_lowering=False, debug=False)

    xt = nc.dram_tensor("xt", [B, KT, 128, S], F32R, kind="ExternalInput")
    wq = nc.dram_tensor("wq", [KT, 128, HPC * DH], F32R, kind="ExternalInput")
    wk = nc.dram_tensor("wk", [KT, 128, HPC * DH], F32R, kind="ExternalInput")
    wv = nc.dram_tensor("wv", [KT, 128, HPC * DH], F32R, kind="ExternalInput")
    wo = nc.dram_tensor("wo", [HPC, 128, D], F32R, kind="ExternalInput")
    bq2 = nc.dram_tensor("bq2", [HPC, 128], F32, kind="ExternalInput")
    bk2 = nc.dram_tensor("bk2", [HPC, 128], F32, kind="ExternalInput")
    mblk = nc.dram_tensor("mblk", [128, 128], F32, kind="ExternalInput")
    onem = nc.dram_tensor("onem", [128, 128], F32R, kind="ExternalInput")
    out = nc.dram_tensor("out", [B, S, D], F32, kind="ExternalOutput")

    with tile.TileContext(nc) as tc:
        with (
            tc.tile_pool(name="const", bufs=1) as constp,
            tc.tile_pool(name="xtp", bufs=20) as xtp,
            tc.tile_pool(name="qkv", bufs=1) as qkvp,
            tc.tile_pool(name="expp", bufs=5) as expp,
            tc.tile_pool(name="attnp", bufs=5) as attnp,
            tc.tile_pool(name="ostp", bufs=6) as ostp,
            # PSUM bank budget (8): st 3 + attn 2 + op 3
            tc.tile_pool(name="ps_st", bufs=3, space="PSUM") as ps_st,
            tc.tile_pool(name="ps_at", bufs=2, space="PSUM") as ps_at,
            tc.tile_pool(name="ps_op", bufs=3, space="PSUM") as ps_op,
        ):
            wq_t = constp.tile([128, KT, HPC * DH], F32R, tag="wq")
            wk_t = constp.tile([128, KT, HPC * DH], F32R, tag="wk")
            wv_t = constp.tile([128, KT, HPC * DH], F32R, tag="wv")
            wo_t = constp.tile([128, HPC, D], F32R, tag="wo")
            for k in range(KT):
                nc.gpsimd.dma_start(wq_t[:, k, :], wq[k])
                nc.gpsimd.dma_start(wk_t[:, k, :], wk[k])
                nc.gpsimd.dma_start(wv_t[:, k, :], wv[k])
            nc.gpsimd.dma_start(wo_t[:], wo.rearrange("h p d -> p h d"))
            bq_t = constp.tile([128, HPC], F32, tag="bq")
            bk_t = constp.tile([128, HPC], F32, tag="bk")
            nc.gpsimd.dma_start(bq_t[:], bq2.rearrange("h p -> p h"))
            nc.gpsimd.dma_start(bk_t[:], bk2.rearrange("h p -> p h"))
            mask_t = constp.tile([128, 128], F32, tag="mask")
            nc.gpsimd.dma_start(mask_t[:], mblk[:])
            ones_m = constp.tile([128, 128], F32R, tag="ones_m")
            nc.gpsimd.dma_start(ones_m[:], onem[:])
            warm_t = constp.tile([128, 1], F32, tag="warm")
            nc.scalar.activation(warm_t[:], mask_t[:, 0:1], AF.Exp, scale=0.0)

            units = []
            for b in range(B):
                # ---------------- projections for batch b ----------------
                qt = qkvp.tile([128, HPC, S], F32R, tag="qt", name=f"qt{b}")
                ktt = qkvp.tile([128, HPC, S], F32R, tag="ktt",
                                name=f"ktt{b}")
                vt = qkvp.tile([128, NST, HPC * DH], F32R, tag="vt",
                               name=f"vt{b}")
                def outproj(qc, at_sb, b=b):
                    for qs in range(4):
                        row0 = qc * 512 + qs * 128
                        for nch in range(4):
                            ps = ps_op.tile([128, 512], F32, tag="ps",
                                            name="op_ps")
                            for h in range(HPC):
                                nc.tensor.matmul(
                                    ps[:],
                                    at_sb[h][:, qs * 128 : (qs + 1) * 128],
                                    wo_t[:, h, nch * 512 : (nch + 1) * 512],
                                    start=(h == 0),
                                    stop=(h == HPC - 1),
                                )
                            o_t = ostp.tile([128, 512], F32, tag="ost",
                                            name="o_t")
                            nc.vector.tensor_copy(o_t[:, :256], ps[:, :256])
                            nc.scalar.copy(o_t[:, 256:], ps[:, 256:])
                            nc.gpsimd.dma_start(
                                out[
                                    b,
                                    row0 : row0 + 128,
                                    nch * 512 : (nch + 1) * 512,
                                ],
                                o_t[:],
                            )


                def p_chunk(c, b=b, qt=qt, ktt=ktt, vt=vt):
                    xts = []
                    for k in range(KT):
                        x_t = xtp.tile([128, 512], F32R, tag="xt",
                                       name="x_t")
                        nc.sync.dma_start(
                            x_t[:], xt[b, k, :, c * 512 : (c + 1) * 512]
                        )
                        xts.append(x_t)
                    for h in range(HPC):
                        for w_t, dst, bias_t, use_act in (
                            (wq_t, qt, bq_t, True),
                            (wk_t, ktt, bk_t, False),
                        ):
                            pool = ps_at if use_act else ps_op
                            ps = pool.tile([128, 512], F32, tag="ps",
                                           name="qk_ps")
                            for k in range(KT):
                                nc.tensor.matmul(
                                    ps[:],
                                    w_t[:, k, h * DH : (h + 1) * DH],
                                    xts[k][:],
                                    start=(k == 0),
                                    stop=(k == KT - 1),
                                )
                            dsl = dst[:, h, c * 512 : (c + 1) * 512]
                            if use_act:
                                nc.scalar.add(dsl, ps[:], bias_t[:, h : h + 1])
                            else:
                                nc.vector.tensor_scalar_add(
                                    dsl, ps[:], bias_t[:, h : h + 1]
                                )
                    for s in range(4):
                        ps = ps_st.tile([128, HPC * DH], F32, tag="st",
                                        name="v_ps")
                        for k in range(KT):
                            nc.tensor.matmul(
                                ps[:],
                                xts[k][:, s * 128 : (s + 1) * 128],
                                wv_t[:, k, :],
                                start=(k == 0),
                                stop=(k == KT - 1),
                            )
                        if s % 2 == 0:
                            nc.vector.tensor_copy(vt[:, c * 4 + s, :], ps[:])
                        else:
                            nc.scalar.copy(vt[:, c * 4 + s, :], ps[:])

                def a_chunk(qc, qt=qt, ktt=ktt, vt=vt):
                    n_kt = 4 * (qc + 1)
                    qsl = slice(qc * 512, (qc + 1) * 512)
                    at_sb = []
                    for h in range(HPC):
                        hsl = slice(h * DH, (h + 1) * DH)
                        attn_ps = ps_at.tile([128, 512], F32, tag="ps",
                                             name="attn_ps")
                        rbden_ps = ps_op.tile([128, 512], F32, tag="ps",
                                              name="rbden_ps")
                        for i in range(n_kt):
                            st = ps_st.tile([128, 512], F32, tag="st",
                                            name="st")
                            expt = expp.tile([128, 512], F32R, tag="exp",
                                             name="expt")
                            lo0 = 128 * (i - 4 * qc) if i >= 4 * qc else 0
                            nc.tensor.matmul(
                                st[:, lo0:],
                                ktt[:, h, i * 128 : (i + 1) * 128],
                                qt[:, h, qc * 512 + lo0 : (qc + 1) * 512],
                                start=True,
                                stop=True,
                            )
                            if i >= 4 * qc:  # diagonal tile
                                lo = 128 * (i - 4 * qc)
                                nc.vector.tensor_tensor(
                                    st[:, lo : lo + 128],
                                    st[:, lo : lo + 128],
                                    mask_t[:],
                                    op=ALU.add,
                                )
                            else:
                                lo = 0
                            nc.scalar.activation(
                                expt[:, lo:], st[:, lo:], AF.Exp, scale=SCALE
                            )
                            nc.tensor.matmul(
                                attn_ps[:, lo:],
                                vt[:, i, hsl],
                                expt[:, lo:],
                                start=(i == 0),
                                stop=(i == n_kt - 1),
                            )
                            nc.tensor.matmul(
                                rbden_ps[:, lo:],
                                ones_m[:],
                                expt[:, lo:],
                                start=(i == 0),
                                stop=(i == n_kt - 1),
                            )
                        rc_sb = attnp.tile([128, 512], F32, tag="rc",
                                           name="rc_sb")
                        nc.vector.reciprocal_approx_fast(
                            out=rc_sb[:], in_=rbden_ps[:]
                        )
                        a_sb = attnp.tile([128, 512], F32R, tag="attnT",
                                          name="a_sb")
                        nc.vector.tensor_tensor(
                            a_sb[:], attn_ps[:], rc_sb[:], op=ALU.mult
                        )
                        at_sb.append(a_sb)
                    return at_sb

                units.append((p_chunk, a_chunk, outproj))

            pending = None
            for p_fn, a_fn, op_fn in units:
                for c in range(NQC):
                    p_fn(c)
                    at = a_fn(c)
                    if pending is not None:
                        pending[0](pending[1], pending[2])
                    pending = (op_fn, c, at)
            pending[0](pending[1], pending[2])
    nc.compile()
    return nc


def _get_nc():
    if "nc" not in _NC_CACHE:
        _NC_CACHE["nc"] = _build()
    return _NC_CACHE["nc"]


def kernel(x, mask, Wq, bq, Wk, bk, Wv, bv, Wo, bo):
    x = np.asarray(x, dtype=np.float32)
    Wq = np.asarray(Wq, dtype=np.float32)
    Wk = np.asarray(Wk, dtype=np.float32)
    Wv = np.asarray(Wv, dtype=np.float32)
    Wo = np.asarray(Wo, dtype=np.float32)
    bq = np.asarray(bq, dtype=np.float32)
    bk = np.asarray(bk, dtype=np.float32)
    bv = np.asarray(bv, dtype=np.float32)
    bo = np.asarray(bo, dtype=np.float32)

    nc = _get_nc()

    xt_np = np.ascontiguousarray(
        x.transpose(0, 2, 1).reshape(B, KT, 128, S)
    )
    kl = np.arange(128)
    mblk = np.where(kl[:, None] <= kl[None, :], 0.0, NEG).astype(np.float32)

    in_maps = []
    for c in range(NCORES):
        cols = slice(c * HPC * DH, (c + 1) * HPC * DH)
        in_maps.append(
            {
                "xt": xt_np,
                "wq": np.ascontiguousarray(Wq[:, cols]).reshape(
                    KT, 128, HPC * DH
                ),
                "wk": np.ascontiguousarray(Wk[:, cols]).reshape(
                    KT, 128, HPC * DH
                ),
                "wv": np.ascontiguousarray(Wv[:, cols]).reshape(
                    KT, 128, HPC * DH
                ),
                "wo": np.ascontiguousarray(Wo[cols, :]).reshape(HPC, 128, D),
                "bq2": np.ascontiguousarray(bq[cols]).reshape(HPC, 128),
                "bk2": np.ascontiguousarray(bk[cols]).reshape(HPC, 128),
                "mblk": mblk,
                "onem": np.ones((128, 128), dtype=np.float32),
            }
        )

    trace = bool(int(os.environ.get("BASS_ATTN_TRACE", "0")))
    try:
        res = run_bass_kernel_spmd(
            nc, in_maps, core_ids=list(range(NCORES)), trace=trace
        )
    except Exception:
        # transient device errors (e.g. a wedged core from a prior run)
        # usually clear on retry
        res = run_bass_kernel_spmd(
            nc, in_maps, core_ids=list(range(NCORES)), trace=trace
        )
    if trace:
        _NC_CACHE["last_result"] = res

    acc = res.results[0]["out"].astype(np.float32)
    for c in range(1, NCORES):
        acc += res.results[c]["out"]
    # bv's effect: softmax rows sum to 1, so attn = attn_nobv + bv per head
    # -> out += bv @ Wo (exact). bo added directly.
    corr = (bv.astype(np.float64) @ Wo.astype(np.float64)) + bo.astype(
        np.float64
    )
    acc += corr.astype(np.float32)
    return acc
